# revision 56
# baseline (speedup 1.0000x reference)
"""Trainium2 Bass kernel for CdfgReader GNN message passing.

Strategy:
  - The GNN node features depend only on which CDFG a batch item references.
    With 64 batch items drawn from 32 CDFGs, compute the GNN once per UNIQUE
    graph (<=32) and distribute 4 graph slots per core across 8 cores.
  - The dominant cost is the A @ (X @ W) multiply per layer. A is a 0/1
    adjacency, exactly representable in fp8e4m3 -- so the A-multiplies run as
    fp8 DoubleRow matmuls (2 contraction rows per partition, 0.5 cycles/row:
    4x the f32r FLOP rate). XW is split on-device into fp8 parts (hi + lo,
    ~8.5 mantissa bits on layers 0-2; hi-only on layer 3); per-layer W
    pre-scaling (2^k, compensated for free in the relu/tanh scale) keeps XW
    away from the fp8 subnormal floor. Split work is spread across Act (hi
    copy), DVE (residual subtract), and gpsimd (lo copy).
  - X@W stays f32r: X activations hi-only (12 bit); W_gcn layers 0-1 split
    hi+lo (22 bit) -- W rounding is the error-critical tensor; W_in and
    W_gcn[2:4] f32r hi-only. Measured end-to-end rel err 1.21e-2 on HW
    (gate 2e-2).
  - Per graph slot: X0 = relu(xs @ W_in) twice (h-major for the GCN stack,
    node-major for the residual), 4 GCN layers, tanh + residual add, then a
    masked-mean mask-matmul (mask pre-divided by its count on the host)
    accumulated across all 4 slots in one PSUM group.
  - Scheduling: graphs run in pairs, layer-by-layer interleaved, so every
    split/relu latency chain of one graph hides behind a full layer of PE
    work from its partner; the next pair's X0 stages, deferred tanh+residual
    batches, and mask-matmuls fill the remaining stall windows. A-multiply
    and X0 psums ride a 4-bank ring (psx), XW psums a 3-bank ring (psw),
    and the cross-graph mask accumulator keeps its own bank. DMAs are
    ordered by first use (the DMA device is serial in the cost model).
  - A^T is pre-transposed/packed to fp8 on the host; biases are all-zero in
    the graded inputs so the zero-bias program is the fast path (a
    bias-capable variant is built lazily if nonzero biases ever show up).
"""

import numpy as np
import ml_dtypes

NG = 4          # graph slots per core
NCORES = 8
N = 1024        # max nodes
F = 128         # input feature dim
H = 256         # hidden dim
L = 4           # GCN layers
B = 64          # batch (coverpoints)

PARTS = (2, 2, 2, 1)            # fp8 split parts per layer's A-multiply
SCALES = (32.0, 8.0, 2.0, 0.5)  # per-layer W_gcn pre-scale (fp8 range centering)
WG_SPLIT = (True, True, False, False)  # W_gcn f32r hi+lo per layer

FP8 = np.dtype(ml_dtypes.float8_e4m3)

_CACHE = {}


def _build_nc(zero_bias=True):
    import concourse.bass as bass  # noqa: F401
    import concourse.mybir as mybir
    import concourse.tile as tile
    from concourse import bacc
    from concourse.bass import ts

    f32 = mybir.dt.float32
    f32r = mybir.dt.float32r
    f8 = mybir.dt.float8e4
    Relu = mybir.ActivationFunctionType.Relu
    Tanh = mybir.ActivationFunctionType.Tanh
    Copy = mybir.ActivationFunctionType.Copy
    sub = mybir.AluOpType.subtract
    mult = mybir.AluOpType.mult
    amax = mybir.AluOpType.max
    DR = mybir.MatmulPerfMode.DoubleRow

    nc = bacc.Bacc("TRN2", target_bir_lowering=False, debug=False,
                   num_devices=NCORES)

    a8_t = nc.dram_tensor("a8_t", [NG, 128, 8, N], f8, kind="ExternalInput")
    xs_t = nc.dram_tensor("xs_t", [128, NG, N], f32r, kind="ExternalInput")
    m_t = nc.dram_tensor("m_t", [128, NG * 8, B], f32r, kind="ExternalInput")
    w_in_hi = nc.dram_tensor("w_in_hi", [F, H], f32r, kind="ExternalInput")
    w_gcn_hi = nc.dram_tensor("w_gcn_hi", [128, L * 2, H], f32r,
                              kind="ExternalInput")
    w_gcn_lo = nc.dram_tensor("w_gcn_lo", [128, L * 2, H], f32r,
                              kind="ExternalInput")
    if not zero_bias:
        b_in_pp = nc.dram_tensor("b_in_pp", [128, 2], f32, kind="ExternalInput")
        b_gcn_pp = nc.dram_tensor("b_gcn_pp", [128, L * 2], f32,
                                  kind="ExternalInput")
        b_in_row = nc.dram_tensor("b_in_row", [1, H], f32r, kind="ExternalInput")
        b_g3_row = nc.dram_tensor("b_g3_row", [1, H], f32r, kind="ExternalInput")
        ones_row = nc.dram_tensor("ones_row", [1, 128], f32r, kind="ExternalInput")
    out = nc.dram_tensor("out", [B, H], f32, kind="ExternalOutput")

    with tile.TileContext(nc) as tc:
        with (
            tc.tile_pool(name="const", bufs=1) as constp,
            tc.tile_pool(name="apool", bufs=2) as apool,
            tc.tile_pool(name="xpool", bufs=2) as xpool,
            tc.tile_pool(name="rpool", bufs=2) as rpool,
            tc.tile_pool(name="psx", bufs=4, space="PSUM") as psx,
            tc.tile_pool(name="psw", bufs=3, space="PSUM") as psw,
            tc.tile_pool(name="psm", bufs=1, space="PSUM") as psm,
        ):
            # --- constants, ordered by first use so the serial DMA stream
            # never blocks the PE: w_in + xs0 first, then layer-0 weights,
            # a8_0, m_t, the remaining layer weights, mask ---
            wi_hi_sb = constp.tile([128, H], f32r)
            nc.sync.dma_start(wi_hi_sb[:], w_in_hi[:, :])
            w_hi_sb = constp.tile([128, L * 2, H], f32r)
            w_lo_sb = constp.tile([128, L * 2, H], f32r)
            m_t_sb = constp.tile([128, NG * 8, B], f32r)

            def emit_wg_dma(layer):
                sl = slice(2 * layer, 2 * layer + 2)
                nc.sync.dma_start(w_hi_sb[:, sl, :], w_gcn_hi[:, sl, :])
                if WG_SPLIT[layer]:
                    nc.sync.dma_start(w_lo_sb[:, sl, :], w_gcn_lo[:, sl, :])
            if not zero_bias:
                b_in_pp_sb = constp.tile([128, 2], f32)
                nc.sync.dma_start(b_in_pp_sb[:], b_in_pp[:, :])
                b_gcn_pp_sb = constp.tile([128, L * 2], f32)
                nc.sync.dma_start(b_gcn_pp_sb[:], b_gcn_pp[:, :])
                b_in_row_sb = constp.tile([1, H], f32r)
                nc.sync.dma_start(b_in_row_sb[:], b_in_row[:, :])
                b_g3_row_sb = constp.tile([1, H], f32r)
                nc.sync.dma_start(b_g3_row_sb[:], b_g3_row[:, :])
                ones_sb = constp.tile([1, 128], f32r)
                nc.sync.dma_start(ones_sb[:], ones_row[:, :])

            pm = psm.tile([B, H], mybir.dt.float32, tag="psm")
            n_pm_mm = NG * 8
            pm_k = [0]

            def pm_mm(lhsT, rhs):
                nc.tensor.matmul(pm[:], lhsT, rhs, start=(pm_k[0] == 0),
                                 stop=(pm_k[0] == n_pm_mm - 1))
                pm_k[0] += 1

            # per-graph persistent state handed between stage closures
            st = {}

            def emit_xs_dma(g, split=False):
                xs = xpool.tile([128, N], f32r, tag="xs", name=f"xs_{g}",
                                bufs=3)
                if split:
                    nc.sync.dma_start(xs[:, 0:512], xs_t[:, g, 0:512])
                    nc.sync.dma_start(xs[:, 512:], xs_t[:, g, 512:])
                else:
                    nc.sync.dma_start(xs[:], xs_t[:, g, :])
                st.setdefault(g, {})["xs"] = xs

            def emit_a8_dma(g, split=False):
                a8 = apool.tile([128, 8, N], f8, tag="a8", name=f"a8_{g}",
                                bufs=3)
                if split:
                    nc.sync.dma_start(a8[:, 0:4, :], a8_t[g, :, 0:4, :])
                else:
                    nc.sync.dma_start(a8[:], a8_t[g])
                st.setdefault(g, {})["a8"] = a8
                return a8

            def emit_x0t(g, pool=None):
                # X0^T h-major f32r; quarter-column psums on the half-banked
                # psx ring; relus alternate DVE/Act per t so consumers' two
                # chunks finish concurrently
                xs = st[g]["xs"]
                x_hi = xpool.tile([128, 2, N], f32r, tag="xh",
                                  name=f"x0t_{g}", bufs=4)
                st[g]["x_hi"] = x_hi
                pool = pool or psx
                for c in range(2):
                    for t in range(2):
                        ps = pool.tile([128, 512], mybir.dt.float32,
                                       tag=pool.name, name="ps_x0t")
                        nc.tensor.matmul(ps[:], wi_hi_sb[:, ts(t, 128)],
                                         xs[:, ts(c, 512)], start=True,
                                         stop=True)
                        if not zero_bias:
                            nc.scalar.activation(x_hi[:, t, ts(c, 512)],
                                                 ps[:], Relu,
                                                 bias=b_in_pp_sb[:, t:t + 1])
                        elif t == 0:
                            nc.vector.tensor_scalar(x_hi[:, t, ts(c, 512)],
                                                    ps[:], 1.0, 0.0, mult,
                                                    amax)
                        else:
                            nc.scalar.activation(x_hi[:, t, ts(c, 512)],
                                                 ps[:], Relu)

            def emit_x0n(g, pool=None):
                # X0 node-major (residual); mask-matmuls deferred to pm fills
                xs = st[g]["xs"]
                x0n = xpool.tile([128, 8, H], f32r, tag="x0n",
                                 name=f"x0n_{g}", bufs=4)
                st[g]["x0n"] = x0n
                pool = pool or psx
                for i in range(4):
                    ps = pool.tile([128, 2, H], mybir.dt.float32,
                                   tag=pool.name, name="ps_x0n")
                    for h2 in range(2):
                        blk = 2 * i + h2
                        nc.tensor.matmul(ps[:, h2, :], xs[:, ts(blk, 128)],
                                         wi_hi_sb[:], start=True,
                                         stop=zero_bias)
                        if not zero_bias:
                            nc.tensor.matmul(ps[:, h2, :], ones_sb[:],
                                             b_in_row_sb[:], start=False,
                                             stop=True)
                    if zero_bias and i % 2 == 0:
                        nc.vector.tensor_scalar(x0n[:, 2 * i:2 * i + 2, :],
                                                ps[:], 1.0, 0.0, mult, amax)
                    else:
                        nc.scalar.activation(x0n[:, 2 * i:2 * i + 2, :],
                                             ps[:], Relu)

            def emit_xf_pm(g):
                xf = st[g]["xf"]
                for c in range(8):
                    pm_mm(m_t_sb[:, g * 8 + c, :], xf[:, c, :])

            def emit_layer(g, layer, fill=None):
                parts = PARTS[layer]
                inv_s = 1.0 / SCALES[layer]
                a8 = st[g]["a8"]
                x_hi = st[g]["x_hi"]
                wparts = (w_hi_sb, w_lo_sb) if WG_SPLIT[layer] else (w_hi_sb,)
                nwmm = 2 * len(wparts)
                # XW = X @ (W_gcn[layer] * scale), split into fp8 parts
                xw8 = [xpool.tile([128, 8, H], f8, tag=f"xw8_{j}",
                                  name=f"xw8_{g}_{layer}_{j}", bufs=3)
                       for j in range(parts)]
                for i in range(4):
                    ps = psw.tile([128, 2, H], mybir.dt.float32, tag="psw")
                    for h2 in range(2):
                        blk = 2 * i + h2
                        k = 0
                        for t in range(2):
                            for w_sb in wparts:
                                nc.tensor.matmul(
                                    ps[:, h2, :], x_hi[:, t, ts(blk, 128)],
                                    w_sb[:, layer * 2 + t, :],
                                    start=(k == 0), stop=(k == nwmm - 1))
                                k += 1
                    pr = slice(2 * i, 2 * i + 2)
                    if parts == 1 and i % 2 == 1:
                        # 1-part layers have no DVE sub chain: alternate the
                        # hi-copy across Act/DVE to halve the c1 queue latency
                        nc.vector.tensor_copy(xw8[0][:, pr, :], ps[:])
                    else:
                        nc.scalar.activation(xw8[0][:, pr, :], ps[:], Copy)
                    if parts >= 2:
                        r1 = rpool.tile([128, 2, H], f32, tag="r1")
                        nc.vector.tensor_tensor(r1[:], ps[:],
                                                xw8[0][:, pr, :], sub)
                        nc.gpsimd.tensor_copy(xw8[1][:, pr, :], r1[:])
                    if parts >= 3:
                        r2 = rpool.tile([128, 2, H], f32, tag="r2")
                        nc.vector.tensor_tensor(r2[:], r1[:],
                                                xw8[1][:, pr, :], sub)
                        nc.scalar.activation(xw8[2][:, pr, :], r2[:], Copy)

                if fill is not None:
                    fill()
                if layer < L - 1:
                    # X_next^T[h, i] (h-major), fp8 DoubleRow A-multiply into
                    # 8 quarter-column psums on the half-banked psx ring
                    xn = xpool.tile([128, 2, N], f32r, tag="xh",
                                    name=f"x{layer + 1}t_{g}", bufs=4)
                    pss = [psx.tile([128, 512], mybir.dt.float32,
                                    tag="psx", name=f"psa_{t}_{c}")
                           for t in range(2) for c in range(2)]
                    # j-outer: the part-0 matmuls only need the Act hi-copy,
                    # so the PE never waits on the slower lo-split chain
                    for j in range(parts):
                        for d in range(4):
                            first = (d == 0 and j == 0)
                            last = (d == 3 and j == parts - 1)
                            for t in range(2):
                                for c in range(2):
                                    nc.tensor.matmul(
                                        pss[t * 2 + c][:],
                                        xw8[j][:, 2 * d:2 * d + 2, ts(t, 128)],
                                        a8[:, 2 * d:2 * d + 2, ts(c, 512)],
                                        start=first, stop=last, perf_mode=DR)
                    # c-major so next layer's first XW pairs unlock first;
                    # t0 on DVE / t1 on Act so both finish concurrently
                    for c in range(2):
                        for t in range(2):
                            if zero_bias and t == 0:
                                nc.vector.tensor_scalar(
                                    xn[:, t, ts(c, 512)], pss[t * 2 + c][:],
                                    inv_s, 0.0, mult, amax)
                            elif zero_bias:
                                nc.scalar.activation(
                                    xn[:, t, ts(c, 512)], pss[t * 2 + c][:],
                                    Relu, scale=inv_s)
                            else:
                                nc.scalar.activation(
                                    xn[:, t, ts(c, 512)], pss[t * 2 + c][:],
                                    Relu,
                                    bias=b_gcn_pp_sb[:, layer * 2 + t:
                                                     layer * 2 + t + 1],
                                    scale=inv_s)
                    st[g]["x_hi"] = xn
                else:
                    # Final layer node-major: 8 per-block psum groups packed
                    # two per bank, d-middle ordering across groups so no
                    # matmul waits on a split. Mask-matmuls deferred.
                    xf = xpool.tile([128, 8, H], f32r, tag="xf",
                                    name=f"xf_{g}")
                    st[g]["xf"] = xf
                    pss3 = []
                    for i in range(4):
                        ps = psx.tile([128, 2, H], mybir.dt.float32,
                                      tag="psx", name=f"ps3_{i}")
                        pss3.append(ps)
                        for h2 in range(2):
                            blk = 2 * i + h2
                            k = 0
                            nmm = 4 * parts
                            for j in range(parts):
                                for d in range(4):
                                    nc.tensor.matmul(
                                        ps[:, h2, :],
                                        a8[:, 2 * d:2 * d + 2, ts(blk, 128)],
                                        xw8[j][:, 2 * d:2 * d + 2, :],
                                        start=(k == 0),
                                        stop=(k == nmm - 1 and zero_bias),
                                        perf_mode=DR)
                                    k += 1
                            if not zero_bias:
                                nc.tensor.matmul(ps[:, h2, :], ones_sb[:],
                                                 b_g3_row_sb[:],
                                                 start=False, stop=True)

                    def post3(g=g, pss3=pss3, xf=xf, inv_s=inv_s):
                        # tanh + residual add, deferred so these Act/DVE ops
                        # don't head-of-line block the next stage's splits
                        for i in range(4):
                            xft = rpool.tile([128, 2, H], f32, tag="xft")
                            nc.scalar.activation(xft[:], pss3[i][:], Tanh,
                                                 scale=inv_s)
                            nc.vector.tensor_tensor(
                                xf[:, 2 * i:2 * i + 2, :], xft[:],
                                st[g]["x0n"][:, 2 * i:2 * i + 2, :],
                                mybir.AluOpType.add)
                    st[g]["post3"] = post3


            # --- two-graph interleaved emission ---
            # Graphs run in pairs, layer-by-layer interleaved: every
            # split/relu latency chain of graph g hides behind a full layer
            # (~7us of PE work) of its partner. DMAs are ordered by first use.
            emit_xs_dma(0, split=True)
            emit_wg_dma(0)
            a8_0 = emit_a8_dma(0, split=True)
            emit_xs_dma(1)
            nc.sync.dma_start(a8_0[:, 4:8, :], a8_t[0, :, 4:8, :])
            emit_x0t(0, pool=psw)
            emit_x0n(0, pool=psx)
            nc.sync.dma_start(m_t_sb[:], m_t[:, :, :])
            for layer in range(1, L):
                emit_wg_dma(layer)
            emit_a8_dma(1)
            emit_x0t(1, pool=psw)
            emit_x0n(1)

            for p in (0, 2):
                g0, g1 = p, p + 1
                last_pair = p + 2 >= NG
                gn0, gn1 = g0 + 2, g1 + 2
                # fills run between a stage's XW and A phases: the psx ring
                # is covered by the XW work, and the fill's relus drain
                # before the A-multiply needs the banks back.
                emit_layer(g0, 0, fill=(
                    (lambda g1=g1: (st[g1 - 2]["post3"](), emit_x0n(g1)))
                    if p > 0 else None))
                emit_layer(g1, 0)
                if not last_pair:
                    emit_xs_dma(gn0)
                    emit_a8_dma(gn0)
                emit_layer(g0, 1)
                emit_layer(g1, 1)
                if not last_pair:
                    emit_xs_dma(gn1)
                    emit_a8_dma(gn1)
                emit_layer(g0, 2)
                emit_layer(g1, 2, fill=(
                    (lambda gn0=gn0: emit_x0t(gn0)) if not last_pair
                    else (lambda: emit_xf_pm(0))))
                emit_layer(g0, 3, fill=(
                    (lambda gn0=gn0: emit_x0n(gn0)) if not last_pair
                    else (lambda: emit_xf_pm(1))))
                emit_layer(g1, 3, fill=(
                    (lambda g0=g0, gn1=gn1:
                     (st[g0]["post3"](), emit_x0t(gn1))) if not last_pair
                    else (lambda g0=g0: st[g0]["post3"]())))
            st[NG - 1]["post3"]()
            emit_xf_pm(NG - 2)
            emit_xf_pm(NG - 1)

            # --- epilogue: mask counts are pre-divided into m_t on host ---
            out_sb = constp.tile([B, H], f32)
            nc.vector.tensor_copy(out_sb[:], pm[:])
            nc.sync.dma_start(out[:, :], out_sb[:])

    nc.compile()
    return nc


def _get_nc(zero_bias=True):
    key = ("nc", zero_bias)
    if key not in _CACHE:
        _CACHE[key] = _build_nc(zero_bias)
    return _CACHE[key]


def _rnd11(x):
    # round-to-nearest-even at f32r precision (11 explicit mantissa bits)
    m, e = np.frexp(np.float32(x))
    m = np.round(m * 4096.0) / 4096.0
    return np.ldexp(m, e).astype(np.float32)


def _prepare_in_maps(cdfg_xs, cdfg_as, graph, coverpoint_mask,
                     W_in, b_in, W_gcn, b_gcn, zero_bias=True):
    cdfg_xs = np.asarray(cdfg_xs, dtype=np.float32)
    cdfg_as = np.asarray(cdfg_as, dtype=np.float32)
    graph = np.asarray(graph).astype(np.int64)
    maskf = np.asarray(coverpoint_mask).astype(np.float32)
    W_in = np.asarray(W_in, dtype=np.float32)
    b_in = np.asarray(b_in, dtype=np.float32)
    W_gcn = np.asarray(W_gcn, dtype=np.float32)
    b_gcn = np.asarray(b_gcn, dtype=np.float32)

    uniq = np.unique(graph)
    nslots = NG * NCORES
    slots = np.empty(nslots, dtype=np.int64)
    slots[:len(uniq)] = uniq
    slots[len(uniq):] = uniq[0]
    real = np.zeros(nslots, dtype=bool)
    real[:len(uniq)] = True

    # W_gcn layout [128, L*2, H], pre-scaled per layer, split hi/lo
    w_gcn_layout = np.ascontiguousarray(
        W_gcn.reshape(L, 2, 128, H).transpose(2, 0, 1, 3)
        .reshape(128, L * 2, H))
    scale_vec = np.repeat(np.asarray(SCALES, np.float32), 2)  # [L*2]
    w_scaled = w_gcn_layout * scale_vec[None, :, None]
    w_gcn_hi = _rnd11(w_scaled)
    w_gcn_lo = np.ascontiguousarray(w_scaled - w_gcn_hi)
    w_in_hi = _rnd11(W_in)

    one8 = np.float32(1.0).astype(FP8).view(np.uint8)

    common = {
        "w_in_hi": np.ascontiguousarray(w_in_hi),
        "w_gcn_hi": np.ascontiguousarray(w_gcn_hi),
        "w_gcn_lo": w_gcn_lo,
    }
    if not zero_bias:
        common.update({
            "b_in_pp": np.ascontiguousarray(b_in.reshape(2, 128).T),
            "b_gcn_pp": np.ascontiguousarray(
                b_gcn.reshape(L, 2, 128).transpose(2, 0, 1).reshape(128, L * 2)),
            "b_in_row": np.ascontiguousarray(b_in.reshape(1, H)),
            "b_g3_row": np.ascontiguousarray(
                (b_gcn[L - 1] * SCALES[L - 1]).reshape(1, H)),
            "ones_row": np.ones((1, 128), dtype=np.float32),
        })

    in_maps = []
    for k in range(NCORES):
        sl = slots[k * NG:(k + 1) * NG]
        a8 = np.empty((NG, 128, 8, N), dtype=np.uint8)
        for g in range(NG):
            at = cdfg_as[sl[g]].T.reshape(8, 128, N).transpose(1, 0, 2)
            a8[g] = (at != 0) * one8
        xs_t = np.ascontiguousarray(cdfg_xs[sl].transpose(2, 0, 1))
        m_t = np.zeros((128, NG * 8, B), dtype=np.float32)
        for g in range(NG):
            if real[k * NG + g]:
                rows = np.nonzero(graph == sl[g])[0]
                for b in rows:
                    m_t[:, g * 8:(g + 1) * 8, b] = (
                        maskf[b] / maskf[b].sum()).reshape(8, 128).T
        in_maps.append({"a8_t": a8.view(FP8), "xs_t": xs_t,
                        "m_t": m_t, **common})
    return in_maps, slots, real


def _assemble_out(results, graph, slots, real):
    graph = np.asarray(graph).astype(np.int64)
    out = np.zeros((B, H), dtype=np.float32)
    for k in range(NCORES):
        for g in range(NG):
            if real[k * NG + g]:
                rows = graph == slots[k * NG + g]
                out[rows] = results[k]["out"][rows]
    return out


def kernel(cdfg_xs, cdfg_as, graph, coverpoint_mask, W_in, b_in, W_gcn, b_gcn):
    from concourse.bass_utils import run_bass_kernel_spmd

    zero_bias = (not np.any(np.asarray(b_in))) and (not np.any(np.asarray(b_gcn)))
    nc = _get_nc(zero_bias)
    in_maps, slots, real = _prepare_in_maps(
        cdfg_xs, cdfg_as, graph, coverpoint_mask, W_in, b_in, W_gcn, b_gcn,
        zero_bias)
    res = run_bass_kernel_spmd(nc, in_maps, core_ids=list(range(NCORES)))
    return _assemble_out(res.results, graph, slots, real)
